# revision 1
# baseline (speedup 1.0000x reference)
"""VQ codebook lookup (DiscreteDecisionEngine) on 8 TRN2 NeuronCores.

Math: for each token x_t, find argmin_k ||x_t - c_k||^2, emit codebook[k].
argmin_k ||x-c||^2 == argmax_k (2*x.c_k - ||c_k||^2)  (||x||^2 constant per token).

Device strategy (data-parallel over tokens, codebook replicated per core):
  - Token tile = 128 tokens. Scores for 8192 codes per tile computed as 4
    PSUM "quarters" of 2048 codes.
  - PE float32r matmuls compute 32*(2 x.c) partial sums only (inputs
    pre-scaled by 8 on each side; no csq rows on the PE at all).
  - One fused custom-DVE op per quarter reads PSUM and computes
      v_k = round(ps_k + ncsq_k - bias_t) + (k_local+1) * 2^-11
    (magic-number rounding to an integer u at 1/32-of-a-score steps, index
    packed in the binary fraction; all exact in fp32 while |u| < 4096),
    writing v to SBUF and accumulating max(v) per quarter (accum=MAX).
    max(v) simultaneously encodes the quarter's best quantized score AND
    its argmax index. bias_t = 32*(a*||x_t|| + b + 512) centers near-max
    scores at 0 (fit for N(0,1) data).
  - ACT counts in-band codes per quarter with a Sign activation on v
    (bias = 14.5 - u_gmax, accum sum): v > u_tau <=> u >= u_tau.
  - decode (per tile, tiny [P,4]/[P,1] ops): u_g = round(gv - 0.5);
    k_local+1 = (gv - u_g)*2048; quarter offset from (v_q >= gv) mask.
    Total count == 1 => exact argmax; count plus u-overflow guards are
    emitted as a per-token flag.
  - GPSIMD indirect DMA gathers codebook rows, HWDGE stores output.

Host: tokens whose flag != 1 (second code within DELTA of the max, ties,
or |u| overflow) are recomputed exactly in float64. Device score error
<= ~0.15 in raw units (f32r matmul ~0.07 + fp16 csq 0.016 + quantization
0.016 + bias hoist 0.016), so DELTA = 14/32 = 0.4375 keeps ~3x margin;
~3-4% of tokens get flagged.
"""

import numpy as np

import concourse.bacc as bacc
import concourse.bass as bass
import concourse.mybir as mybir
from concourse.tile import TileContext
from concourse.dve_spec import Spec, Src0, Src1, C0, C1, C2, AluOp, lower, scan
from concourse.dve_uop import DveOpSpec
from concourse import dve_ops as _dvo

P = 128          # partitions / token tile
D = 512          # latent dim
K = 8192         # codebook size
N_TOKENS = 32768
N_CORES = 8
T_PER_CORE = N_TOKENS // N_CORES   # 4096
N_TILES_FULL = T_PER_CORE // P     # 32
QUARTER_FULL = 2048                # codes per PSUM quarter (4 banks)
N_CHUNK = D // P                   # 4 contraction chunks
MM_N = 512                         # matmul free-dim block (1 PSUM bank, fp32)

F32 = mybir.dt.float32
F32R = mybir.dt.float32r
F16 = mybir.dt.float16
SCALE = 32.0                       # score scale on device (8x * 8c = 64xc)
DELTA_U = 14.0                     # at-risk band in u units (0.4375 raw)
MAGIC = 12582912.0                 # 1.5 * 2^23: fp32 round-to-int constant
FRAC = 2.0 ** -11                  # index packing step
CSQ_CENTER = 512.0                 # csq recentering constant
BIAS_ALPHA = 5.67992491            # m_hat = alpha*||x|| + beta (N(0,1) fit)
BIAS_BETA = -443.31968665
U_GUARD = 2047.0                   # |u| >= this => decode edge risk => flag
HALFC = 0.500244140625             # 0.5 + 2^-12: floor via round(gv - HALFC)


def _ref_pack_argmax(in0, in1, c0, c1, c2):
    s = in0.astype(np.float32) + in1.astype(np.float32) - c0
    u = np.rint(s).astype(np.float32)
    idx = ((np.arange(in0.shape[-1]) + 1) * c2).astype(np.float32)
    b = u + idx[None, :]
    return b, b.reshape(b.shape[0], -1).max(axis=-1, keepdims=True)


def _get_pack_op():
    name = "VQ_PACK_ARGMAX_ANT"
    for op in _dvo.OPS:
        if op.name == name:
            return op
    body = ((Src0 + Src1 + (C1 - C0)) - C1) + scan(AluOp.ADD, C2)
    spec = Spec(body=body, accum=AluOp.MAX, reference=_ref_pack_argmax)
    row = max(_dvo._SUB_OPCODE_FOR_NAME.values()) + 1
    assert row < 0x20
    shas = {}
    for ver in ("v3", "v4"):
        uops = lower(spec, ver=ver)
        shas[ver] = DveOpSpec(name=name, opcode=row, uops=uops,
                              rd1_en=True).sha(ver)
    op = _dvo.DveOp(name, spec, subdim=False, uops_sha=shas)
    _dvo.OPS.append(op)
    _dvo.CUSTOM_DVE_SPECS[name] = spec
    _dvo._SUB_OPCODE_FOR_NAME[name] = row
    return op


def build_bass(n_tiles=N_TILES_FULL, k=K, quarter=QUARTER_FULL, repeat=1):
    """Build the single-core Bass program (SPMD across cores)."""
    pack_op = _get_pack_op()
    n_q = k // quarter
    n_sb = max(1, quarter // MM_N)
    sb = min(MM_N, quarter)

    nc = bacc.Bacc()
    x_tiles = nc.declare_dram_parameter(
        "x_tiles", [n_tiles, P, N_CHUNK, P], F32R, isOutput=False)
    cb_tiles = nc.declare_dram_parameter(
        "cb_tiles", [N_CHUNK, n_q, P, quarter], F32R, isOutput=False)
    negcsq16 = nc.declare_dram_parameter(
        "negcsq16", [n_q, P, quarter], F16, isOutput=False)
    iota_nq = nc.declare_dram_parameter(
        "iota_nq", [P, 8 * n_q], F32, isOutput=False)
    bias_in = nc.declare_dram_parameter("bias_in", [P, n_tiles], F32,
                                        isOutput=False)
    magic_in = nc.declare_dram_parameter("magic_in", [P, 1], F32,
                                         isOutput=False)
    codebook = nc.declare_dram_parameter("codebook", [k, D], F32,
                                         isOutput=False)
    out = nc.declare_dram_parameter("out", [n_tiles * P, D], F32,
                                    isOutput=True)
    out_flags = nc.declare_dram_parameter(
        "out_flags", [P, n_tiles], F32, isOutput=True)

    with TileContext(nc) as tc:
        with (
            tc.tile_pool(name="const", bufs=1) as cpool,
            tc.tile_pool(name="xp", bufs=2) as xpool,
            tc.tile_pool(name="sp", bufs=4) as spool,
            tc.tile_pool(name="scr", bufs=1) as scrpool,
            tc.tile_pool(name="small", bufs=2) as smpool,
            tc.tile_pool(name="sm1", bufs=1) as sm1pool,
            tc.tile_pool(name="op", bufs=2) as opool,
            tc.tile_pool(name="ps", bufs=2, space="PSUM") as pspool,
        ):
            # --- resident constants ------------------------------------------
            iota_nq_sb = cpool.tile([P, 8 * n_q], F32, tag="iota_nq")
            nc.scalar.dma_start(out=iota_nq_sb, in_=iota_nq[:, :])
            bias_sb = cpool.tile([P, n_tiles], F32, tag="bias")
            nc.scalar.dma_start(out=bias_sb, in_=bias_in[:, :])
            magic_sb = cpool.tile([P, 1], F32, tag="magic")
            nc.scalar.dma_start(out=magic_sb, in_=magic_in[:, :])
            flags_sb = cpool.tile([P, n_tiles], F32, tag="flags")
            ncsq_sb = {}
            for q in range(n_q):
                t = cpool.tile([P, quarter], F16, tag=f"ncsq_{q}")
                nc.scalar.dma_start(out=t, in_=negcsq16[q])
                ncsq_sb[q] = t
            # fine-grained [P, sb] codebook tiles across the three DMA issuers
            cb_sb = {}
            dma_engs = [nc.sync, nc.scalar, nc.gpsimd]
            for j, (q, s, c) in enumerate(
                    (q, s, c) for q in range(n_q) for s in range(n_sb)
                    for c in range(N_CHUNK)):
                t = cpool.tile([P, sb], F32R, tag=f"cb_{c}_{q}_{s}")
                dma_engs[j % 3].dma_start(
                    out=t, in_=cb_tiles[c, q][:, s * sb:(s + 1) * sb])
                cb_sb[c, q, s] = t

            # --- main loop over token tiles ----------------------------------
            # ACT count passes for tile t are issued interleaved into tile
            # t+1's quarter loop so the PSUM-releasing stage ops stay at the
            # front of the DVE queue. `pending` carries tile t's staged v
            # tiles and count threshold.
            tts = [t for _ in range(repeat) for t in range(n_tiles)]

            def issue_count(pend, q):
                # in-band count on ACT: sum of sign(v - (u_tau - 0.5))
                scr = scrpool.tile([P, quarter], F16, tag="scr")
                nc.scalar.activation(
                    out=scr, in_=pend["staged"][q],
                    func=mybir.ActivationFunctionType.Sign,
                    bias=pend["negvtau"],
                    accum_out=pend["cnt4"][:, q:q + 1])

            def finish_tile(pend):
                # flag = count + overflow guards; idx from packed gv
                tt = pend["tt"]
                cnt4 = pend["cnt4"]
                gv = pend["gv"]
                u_g = pend["u_g"]
                cntsum = sm1pool.tile([P, 1], F32, tag="cntsum")
                nc.vector.reduce_sum(
                    out=cntsum, in_=cnt4, axis=mybir.AxisListType.X)
                half = sm1pool.tile([P, 1], F32, tag="half")
                nc.vector.tensor_scalar_mul(half, cntsum, 0.5)
                # overflow guards: |u_g| >= U_GUARD makes packing inexact
                gpos = sm1pool.tile([P, 1], F32, tag="gpos")
                nc.vector.scalar_tensor_tensor(
                    out=gpos, in0=u_g, scalar=U_GUARD,
                    in1=pend["big"], op0=mybir.AluOpType.is_ge,
                    op1=mybir.AluOpType.mult)
                gneg = sm1pool.tile([P, 1], F32, tag="gneg")
                nc.vector.scalar_tensor_tensor(
                    out=gneg, in0=u_g, scalar=-U_GUARD,
                    in1=pend["big"], op0=mybir.AluOpType.is_le,
                    op1=mybir.AluOpType.mult)
                guard = sm1pool.tile([P, 1], F32, tag="guard")
                nc.vector.tensor_add(guard, gpos, gneg)
                fsum = sm1pool.tile([P, 1], F32, tag="fsum")
                nc.vector.tensor_add(fsum, half, guard)
                nc.vector.tensor_scalar_add(
                    flags_sb[:, tt % n_tiles:tt % n_tiles + 1], fsum,
                    float(n_q * quarter) / 2.0)
                # k_local + 1 = (gv - u_g) * 2048
                kdiff = sm1pool.tile([P, 1], F32, tag="kdiff")
                nc.vector.tensor_sub(kdiff, gv, u_g)
                k1 = sm1pool.tile([P, 1], F32, tag="k1")
                nc.vector.tensor_scalar_mul(k1, kdiff, 2048.0)
                # quarter offset: sum (v_q >= gv) * (2048*q)
                qmask = sm1pool.tile([P, n_q], F32, tag="qmask")
                nc.vector.scalar_tensor_tensor(
                    out=qmask, in0=pend["qv"], scalar=gv,
                    in1=iota_nq_sb[:, :n_q],
                    op0=mybir.AluOpType.is_ge,
                    op1=mybir.AluOpType.mult)
                qoff = sm1pool.tile([P, 1], F32, tag="qoff")
                nc.vector.reduce_sum(
                    out=qoff, in_=qmask, axis=mybir.AxisListType.X)
                idxf = sm1pool.tile([P, 1], F32, tag="idxf")
                nc.vector.tensor_add(idxf, k1, qoff)
                idxm = sm1pool.tile([P, 1], F32, tag="idxm")
                nc.vector.tensor_scalar_add(idxm, idxf, -1.0)
                idxc = sm1pool.tile([P, 1], F32, tag="idxc")
                nc.vector.tensor_scalar_min(idxc, idxm, float(k - 1))
                idxc2 = sm1pool.tile([P, 1], F32, tag="idxc2")
                nc.vector.tensor_scalar_max(idxc2, idxc, 0.0)
                idxu = sm1pool.tile([P, 1], mybir.dt.uint32, tag="idxu")
                nc.vector.tensor_copy(idxu, idxc2)
                # gather codebook row per token and store
                rows = opool.tile([P, D], F32, tag="rows")
                nc.gpsimd.indirect_dma_start(
                    out=rows,
                    out_offset=None,
                    in_=codebook[:, :],
                    in_offset=bass.IndirectOffsetOnAxis(ap=idxu, axis=0),
                )
                nc.sync.dma_start(
                    out=out[(tt % n_tiles) * P:(tt % n_tiles + 1) * P, :],
                    in_=rows)

            big_sb = cpool.tile([P, 1], F32, tag="big")
            nc.vector.memset(big_sb, 1.0e6)

            pending = None
            for tt in tts:
                xt = xpool.tile([P, N_CHUNK, P], F32R, tag="xt")
                nc.sync.dma_start(out=xt, in_=x_tiles[tt % n_tiles])

                qv = smpool.tile([P, n_q], F32, tag="qv")
                staged = []
                for q in range(n_q):
                    ps = pspool.tile([P, quarter], F32, tag="ps")
                    for c in range(N_CHUNK):
                        for s in range(n_sb):
                            nc.tensor.matmul(
                                out=ps[:, s * sb:(s + 1) * sb],
                                lhsT=xt[:, c, :],
                                rhs=cb_sb[c, q, s][:, :],
                                start=(c == 0),
                                stop=(c == N_CHUNK - 1),
                            )
                    # prev tile's count pass first: traces its read of the
                    # staged buffer BEFORE the pool recycles it for v32 below
                    if pending is not None:
                        issue_count(pending, q)
                    # fused stage: v = round(ps + ncsq - bias) + (k+1)*2^-11
                    v32 = spool.tile([P, quarter], F32, tag="v32")
                    nc.vector._custom_dve(
                        pack_op,
                        out=v32,
                        in0=ps,
                        in1=ncsq_sb[q],
                        s0=bias_sb[:, tt % n_tiles:tt % n_tiles + 1],
                        s1=magic_sb,
                        imm2=FRAC,
                        accum_out=qv[:, q:q + 1],
                    )
                    staged.append(v32)
                if pending is not None:
                    finish_tile(pending)
                    pending = None

                # u_g = round(gv - 0.5); negvtau = (DELTA_U + 0.5) - u_g
                gv = smpool.tile([P, 1], F32, tag="gv")
                nc.vector.reduce_max(
                    out=gv, in_=qv, axis=mybir.AxisListType.X)
                t0 = sm1pool.tile([P, 1], F32, tag="t0")
                nc.vector.tensor_scalar_add(t0, gv, -HALFC)
                t1 = sm1pool.tile([P, 1], F32, tag="t1")
                nc.vector.tensor_scalar_add(t1, t0, MAGIC)
                u_g = smpool.tile([P, 1], F32, tag="u_g")
                nc.vector.tensor_scalar_add(u_g, t1, -MAGIC)
                nvt = smpool.tile([P, 1], F32, tag="nvt")
                nc.vector.tensor_scalar_mul(nvt, u_g, -1.0)
                negvtau = smpool.tile([P, 1], F32, tag="negvtau")
                nc.vector.tensor_scalar_add(negvtau, nvt, DELTA_U + 0.5)
                cnt4 = smpool.tile([P, n_q], F32, tag="cnt4")
                pending = {"staged": staged, "negvtau": negvtau,
                           "cnt4": cnt4, "qv": qv, "gv": gv, "u_g": u_g,
                           "tt": tt, "big": big_sb}

            for q in range(n_q):
                issue_count(pending, q)
            finish_tile(pending)
            pending = None

            nc.sync.dma_start(out=out_flags[:, :], in_=flags_sb)

    return nc


def tf32_round(a, mant=11):
    """Round fp32 to `mant` explicit mantissa bits (round-to-nearest)."""
    ai = a.view(np.int32).astype(np.int64)
    shift = 23 - mant
    bias = 1 << (shift - 1)
    r = ((ai + bias) >> shift) << shift
    return r.astype(np.int32).view(np.float32)


def prep_core_inputs(x_core, shared, n_tiles):
    """Per-core input map. x_core: [n_tiles*P, D]."""
    xt = tf32_round(np.ascontiguousarray(
        (8.0 * x_core).reshape(n_tiles, P, N_CHUNK, P).transpose(0, 3, 2, 1)))
    xn = np.linalg.norm(x_core.astype(np.float64), axis=1)
    bias = (SCALE * (BIAS_ALPHA * xn + BIAS_BETA + CSQ_CENTER)
            ).astype(np.float32)
    bias_pt = np.ascontiguousarray(bias.reshape(n_tiles, P).T)
    return {"x_tiles": xt, "bias_in": bias_pt, **shared}


def prep_shared(codebook, k, quarter):
    n_q = k // quarter
    cb = np.ascontiguousarray(np.asarray(codebook, dtype=np.float32))
    cb8 = 8.0 * cb  # exact in fp32
    cb_tiles_np = tf32_round(np.ascontiguousarray(
        cb8.reshape(n_q, quarter, N_CHUNK, P).transpose(2, 0, 3, 1)))
    csq = (cb.astype(np.float64) ** 2).sum(axis=1)
    ncsq = (SCALE * (CSQ_CENTER - csq)).astype(np.float16)
    negcsq16 = np.ascontiguousarray(np.broadcast_to(
        ncsq.reshape(n_q, 1, quarter), (n_q, P, quarter)))
    offs = np.tile(np.arange(n_q, dtype=np.float32) * quarter, 8)
    iota_nq_np = np.broadcast_to(offs[None, :], (P, len(offs))).copy()
    magic_np = np.full((P, 1), MAGIC, dtype=np.float32)
    return {
        "cb_tiles": cb_tiles_np,
        "negcsq16": negcsq16,
        "iota_nq": iota_nq_np,
        "magic_in": magic_np,
        "codebook": cb,
    }


_NC_CACHE = {}


def _get_nc(key):
    if key not in _NC_CACHE:
        nc = build_bass(*key)
        nc.finalize()
        _NC_CACHE[key] = nc
    return _NC_CACHE[key]


def _host_rescue(out_full, flags_full, x, codebook):
    """Recompute flagged tokens exactly (float64)."""
    bad = np.flatnonzero(flags_full != 1.0)
    if len(bad) == 0:
        return out_full, 0
    xb = x[bad].astype(np.float64)
    cb64 = codebook.astype(np.float64)
    csq = (cb64 * cb64).sum(1)
    sc = 2.0 * (xb @ cb64.T) - csq[None, :]
    idx = sc.argmax(1)
    out_full[bad] = codebook[idx]
    return out_full, len(bad)


def kernel(x, codebook):
    from concourse.bass_utils import run_bass_kernel_spmd

    x = np.ascontiguousarray(np.asarray(x, dtype=np.float32))
    codebook = np.ascontiguousarray(np.asarray(codebook, dtype=np.float32))
    assert x.shape == (N_TOKENS, D) and codebook.shape == (K, D)

    nc = _get_nc((N_TILES_FULL, K, QUARTER_FULL))
    shared = prep_shared(codebook, K, QUARTER_FULL)

    in_maps = []
    for core in range(N_CORES):
        x_core = x[core * T_PER_CORE:(core + 1) * T_PER_CORE]
        in_maps.append(prep_core_inputs(x_core, shared, N_TILES_FULL))

    res = run_bass_kernel_spmd(nc, in_maps, list(range(N_CORES)))
    out_full = np.concatenate(
        [res.results[i]["out"] for i in range(N_CORES)], axis=0)
    flags_full = np.concatenate(
        [np.asarray(res.results[i]["out_flags"]).T.reshape(-1)
         for i in range(N_CORES)])
    out_full, n_rescued = _host_rescue(out_full, flags_full, x, codebook)
    return out_full

